# revision 7
# baseline (speedup 1.0000x reference)
"""Trainium2 Bass kernel for the MHSA bottleneck block.

Contract: kernel(**inputs) takes the FULL unsharded inputs (as produced by
setup_inputs()) and returns the FULL [64, 2048, 14, 14] float32 output.
Internally shards data-parallel over batch: 8 images per NeuronCore, 8 cores.
"""
import sys

sys.path.insert(0, '/opt/trn_rl_repo')

import numpy as np

# Problem constants (hardcoded per the harness contract).
B, CIN, P, H, W = 64, 2048, 512, 14, 14
EPS = 1e-5
N = H * W            # 196 pixels
NCORES = 8
BPC = B // NCORES    # 8 images per core
NPAIR = BPC // 2     # 4 image pairs per core
NPAD = 256           # padded free dim for fp32r full-rate matmuls
KC1 = CIN // 128     # 16 input-channel chunks for conv1 / output chunks conv3
PC = P // 128        # 4 chunks of the 512-dim
N2 = 2 * N           # 392 = free dim for image-pair matmuls

# n/m chunking of the 196-pixel dim: 128 + 68
NCHUNKS = [(0, 128), (128, 68)]

_CACHE = {}


def _build(repeat=1):
    import concourse.bass as bass  # noqa: F401
    import concourse.mybir as mybir
    import concourse.tile as tile
    from concourse import bacc
    from concourse.masks import make_identity

    f32 = mybir.dt.float32
    f32r = mybir.dt.float32r

    nc = bacc.Bacc(None, target_bir_lowering=False, debug=False)

    # DRAM parameters. Matmul operands are declared float32r (same 32-bit
    # storage; the PE rounds internally) so the DMA'd tiles are legal fp32r
    # matmul inputs.
    x_d = nc.declare_dram_parameter("x", [KC1, 128, BPC * N], f32r, isOutput=False)
    w1t_d = nc.declare_dram_parameter("w1t", [KC1, 128, P], f32r, isOutput=False)
    wqkt_d = nc.declare_dram_parameter("wqkt", [PC, 128, 2 * P], f32r, isOutput=False)
    wvt_d = nc.declare_dram_parameter("wvt", [PC, 128, P], f32r, isOutput=False)
    w3t_d = nc.declare_dram_parameter("w3t", [PC, 128, CIN], f32r, isOutput=False)
    pos_d = nc.declare_dram_parameter("pos", [PC, 128, N], f32r, isOutput=False)
    t1_d = nc.declare_dram_parameter("t1", [128, PC], f32, isOutput=False)
    s2_d = nc.declare_dram_parameter("s2", [128, PC], f32, isOutput=False)
    t2_d = nc.declare_dram_parameter("t2", [128, PC], f32, isOutput=False)
    t3_d = nc.declare_dram_parameter("t3", [128, KC1], f32, isOutput=False)
    y_d = nc.declare_dram_parameter("y", [KC1, 128, BPC * N], f32, isOutput=True)

    with tile.TileContext(nc) as tc:
        with (
            tc.tile_pool(name="const", bufs=1) as const,
            tc.tile_pool(name="xp", bufs=2) as xp,
            tc.tile_pool(name="h1p", bufs=1) as h1p,
            tc.tile_pool(name="qkp", bufs=1) as qkp,
            tc.tile_pool(name="h2p", bufs=1) as h2p,
            tc.tile_pool(name="attp", bufs=2) as attp,
            tc.tile_pool(name="outp", bufs=3) as outp,
            tc.tile_pool(name="ps_mm", bufs=3, space="PSUM") as ps_mm,
            tc.tile_pool(name="ps_sm", bufs=4, space="PSUM") as ps_sm,
            tc.tile_pool(name="ps_tr", bufs=1, space="PSUM") as ps_tr,
        ):
            # ---- constants / weights (loaded once) ----
            w1t = const.tile([128, KC1, P], f32r)
            nc.sync.dma_start(out=w1t, in_=x_dma_rearr(w1t_d))
            wqkt = const.tile([128, PC, 2 * P], f32r)
            nc.sync.dma_start(out=wqkt, in_=x_dma_rearr(wqkt_d))
            wvt = const.tile([128, PC, P], f32r)
            nc.sync.dma_start(out=wvt, in_=x_dma_rearr(wvt_d))
            w3t = const.tile([128, PC, CIN], f32r)
            nc.sync.dma_start(out=w3t, in_=x_dma_rearr(w3t_d))
            pos = const.tile([128, PC, N], f32r)
            nc.sync.dma_start(out=pos, in_=x_dma_rearr(pos_d))
            t1 = const.tile([128, PC], f32)
            nc.sync.dma_start(out=t1, in_=t1_d[:, :])
            s2 = const.tile([128, PC], f32)
            nc.sync.dma_start(out=s2, in_=s2_d[:, :])
            t2 = const.tile([128, PC], f32)
            nc.sync.dma_start(out=t2, in_=t2_d[:, :])
            t3 = const.tile([128, KC1], f32)
            nc.sync.dma_start(out=t3, in_=t3_d[:, :])
            ident = const.tile([128, 128], f32)
            make_identity(nc, ident)

            Exp = mybir.ActivationFunctionType.Exp
            Relu = mybir.ActivationFunctionType.Relu
            Copy = mybir.ActivationFunctionType.Copy

            import contextlib
            loop_cm = (tc.For_i(0, repeat, 1) if repeat > 1
                       else contextlib.nullcontext())
            with loop_cm:
              for pair in range(NPAIR):
                nsl = slice(pair * N2, (pair + 1) * N2)

                # ---- load x for this pair: [128, 16, 2*196] fp32(r) ----
                x_t = xp.tile([128, KC1, N2], f32r, name=f"x_{pair}", tag="x")
                for kq in range(4):
                    nc.sync.dma_start(
                        out=x_t[:, kq * 4:(kq + 1) * 4, :],
                        in_=x_d[kq * 4:(kq + 1) * 4, :, nsl].rearrange(
                            "k p n -> p k n"),
                    )
                x_f = x_t.bitcast(f32)

                # ---- conv1 + bn1 + relu -> h1 [128, 4, 392] ----
                h1 = h1p.tile([128, PC, N2], f32r, name=f"h1_{pair}", tag="h1")
                for oc in range(PC):
                    cps = ps_mm.tile([128, 512], f32, name="cps", tag="mm")
                    for kc in range(KC1):
                        nc.tensor.matmul(
                            cps[:, :N2],
                            w1t[:, kc, oc * 128:(oc + 1) * 128],
                            x_t[:, kc, :],
                            start=(kc == 0), stop=(kc == KC1 - 1),
                        )
                    nc.scalar.activation(h1[:, oc, :], cps[:, :N2], Relu,
                                         bias=t1[:, oc:oc + 1])

                # ---- q/k projection -> q_sb/k_sb [128, 4, 2, 256] (padded) ----
                q_sb = qkp.tile([128, PC, 2, NPAD], f32r, name=f"q_{pair}", tag="q")
                k_sb = qkp.tile([128, PC, 2, NPAD], f32r, name=f"k_{pair}", tag="k")
                nc.vector.memset(q_sb.bitcast(f32)[:, :, :, N:], 0.0)
                nc.vector.memset(k_sb.bitcast(f32)[:, :, :, N:], 0.0)
                for oc in range(2 * PC):
                    qps = ps_mm.tile([128, 512], f32, name="qps", tag="mm")
                    for pc in range(PC):
                        nc.tensor.matmul(
                            qps[:, :N2],
                            wqkt[:, pc, oc * 128:(oc + 1) * 128],
                            h1[:, pc, :],
                            start=(pc == 0), stop=(pc == PC - 1),
                        )
                    dst = q_sb if oc < PC else k_sb
                    c4 = oc % PC
                    for j in range(2):
                        nc.vector.tensor_copy(
                            dst[:, c4, j, :N], qps[:, j * N:(j + 1) * N])

                # ---- per-image attention ----
                vT_list = []
                attnT_list = []
                for j in range(2):
                    # v^T directly: vT[m, c] = sum_p h1[p, m] wvt[p, c]
                    vT = attp.tile([128, 2, P], f32r, name=f"vT_{pair}_{j}",
                                   tag="vT")
                    for mi, (m0, msz) in enumerate(NCHUNKS):
                        vps = ps_mm.tile([128, 512], f32, name="vps", tag="mm")
                        for pc in range(PC):
                            nc.tensor.matmul(
                                vps[:msz, :],
                                h1[:, pc, j * N + m0:j * N + m0 + msz],
                                wvt[:, pc, :],
                                start=(pc == 0), stop=(pc == PC - 1),
                            )
                        nc.vector.tensor_copy(vT[:msz, mi, :], vps[:msz, :])

                    # attn^T [128, 2, 256] fp32r (padded cols zeroed)
                    attnT = attp.tile([128, 2, NPAD], f32r,
                                      name=f"aT_{pair}_{j}", tag="attnT")
                    nc.vector.memset(attnT.bitcast(f32)[:, :, N:], 0.0)

                    for ni, (n0, nsz) in enumerate(NCHUNKS):
                        lps = ps_sm.tile([128, NPAD], f32, name="lps",
                                         tag="small")
                        # cc: sum_c q[c, n-slice]^T k[c, :]
                        for pc in range(PC):
                            nc.tensor.matmul(
                                lps[:nsz, :],
                                q_sb[:, pc, j, n0:n0 + nsz],
                                k_sb[:, pc, j, :],
                                start=(pc == 0), stop=False,
                            )
                        # cp: sum_c pos[c, n-slice]^T q[c, :]
                        for pc in range(PC):
                            nc.tensor.matmul(
                                lps[:nsz, :],
                                pos[:, pc, n0:n0 + nsz],
                                q_sb[:, pc, j, :],
                                start=False, stop=(pc == PC - 1),
                            )
                        # softmax over free dim (no max-subtraction needed;
                        # logits are O(40) max, exp stays finite in fp32)
                        p_raw = attp.tile([128, N], f32, name="p_raw",
                                          tag="p_raw")
                        ssum = attp.tile([128, 1], f32, name="ssum", tag="ss")
                        nc.scalar.activation(p_raw[:nsz, :], lps[:nsz, :N],
                                             Exp, accum_out=ssum[:nsz, :])
                        rsum = attp.tile([128, 1], f32, name="rsum", tag="rs")
                        nc.vector.reciprocal(rsum[:nsz, :], ssum[:nsz, :])
                        p_nrm = attp.tile([128, N], f32, name="p_nrm",
                                          tag="p_nrm")
                        nc.vector.tensor_scalar_mul(p_nrm[:nsz, :],
                                                    p_raw[:nsz, :],
                                                    rsum[:nsz, :])
                        # transpose normalized attn into attnT[m, n-slice]
                        for mi, (m0, msz) in enumerate(NCHUNKS):
                            tps = ps_tr.tile([128, 128], f32, name="tps",
                                             tag="tr")
                            nc.tensor.transpose(tps[:msz, :nsz],
                                                p_nrm[:nsz, m0:m0 + msz],
                                                ident[:nsz, :nsz])
                            nc.scalar.activation(attnT[:msz, mi, n0:n0 + nsz],
                                                 tps[:msz, :nsz], Copy)
                    vT_list.append(vT)
                    attnT_list.append(attnT)

                # ---- attention output + bn2 + relu -> h2 [128, 4, 2, 196] ----
                h2 = h2p.tile([128, PC, 2, N], f32r, name=f"h2_{pair}", tag="h2")
                for j in range(2):
                    vT = vT_list[j]
                    attnT = attnT_list[j]
                    for c4 in range(PC):
                        aps = ps_sm.tile([128, NPAD], f32, name="aps",
                                         tag="small")
                        for mi, (m0, msz) in enumerate(NCHUNKS):
                            nc.tensor.matmul(
                                aps[:, :],
                                vT[:msz, mi, c4 * 128:(c4 + 1) * 128],
                                attnT[:msz, mi, :],
                                start=(mi == 0), stop=(mi == 1),
                            )
                        nc.scalar.activation(h2[:, c4, j, :], aps[:, :N],
                                             Relu, bias=t2[:, c4:c4 + 1],
                                             scale=s2[:, c4:c4 + 1])

                # ---- conv3 + bn3 + residual + relu -> y ----
                for oc in range(KC1):
                    ops = ps_mm.tile([128, 512], f32, name="ops", tag="mm")
                    for pc in range(PC):
                        nc.tensor.matmul(
                            ops[:, :N2],
                            w3t[:, pc, oc * 128:(oc + 1) * 128],
                            h2[:, pc, :, :],
                            start=(pc == 0), stop=(pc == PC - 1),
                        )
                    tmp = outp.tile([128, N2], f32, name="tmp", tag="tmp")
                    # tmp = (conv3 + t3) + x
                    nc.vector.scalar_tensor_tensor(
                        tmp, ops[:, :N2], t3[:, oc:oc + 1], x_f[:, oc, :],
                        op0=mybir.AluOpType.add, op1=mybir.AluOpType.add)
                    y_sb = outp.tile([128, N2], f32, name="y_sb", tag="y_sb")
                    nc.scalar.activation(y_sb, tmp, Relu)
                    nc.sync.dma_start(out=y_d[oc, :, nsl], in_=y_sb)

    nc.compile()
    return nc


def x_dma_rearr(d):
    return d[:, :, :].rearrange("k p o -> p k o")


def _prep_inputs(x, w1, g1, b1, m1, v1, wqkv, rel_h, rel_w,
                 g2, b2, m2, v2, w3, g3, b3, m3, v3):
    f = np.float32
    x = np.ascontiguousarray(x, f)
    s1 = (g1 / np.sqrt(v1 + EPS)).astype(f)
    t1 = (b1 - m1 * s1).astype(f)
    s2 = (g2 / np.sqrt(v2 + EPS)).astype(f)
    t2 = (b2 - m2 * s2).astype(f)
    s3 = (g3 / np.sqrt(v3 + EPS)).astype(f)
    t3 = (b3 - m3 * s3).astype(f)

    w1p = (w1 * s1[:, None]).astype(f)                    # [512, 2048]
    w1t = np.ascontiguousarray(w1p.T).reshape(KC1, 128, P)
    wqk = wqkv[:2 * P].astype(f)                          # [1024, 512]
    wqkt = np.ascontiguousarray(wqk.T).reshape(PC, 128, 2 * P)
    wv = wqkv[2 * P:].astype(f)                           # [512, 512]
    wvt = np.ascontiguousarray(wv.T).reshape(PC, 128, P)
    w3p = (w3 * s3[:, None]).astype(f)                    # [2048, 512]
    w3t = np.ascontiguousarray(w3p.T).reshape(PC, 128, CIN)
    pos = (rel_h + rel_w).reshape(P, N).astype(f).reshape(PC, 128, N)

    t1_h = np.ascontiguousarray(t1.reshape(PC, 128).T)
    s2_h = np.ascontiguousarray(s2.reshape(PC, 128).T)
    t2_h = np.ascontiguousarray(t2.reshape(PC, 128).T)
    t3_h = np.ascontiguousarray(t3.reshape(KC1, 128).T)

    shared = dict(w1t=w1t, wqkt=wqkt, wvt=wvt, w3t=w3t, pos=pos,
                  t1=t1_h, s2=s2_h, t2=t2_h, t3=t3_h)

    in_maps = []
    for c in range(NCORES):
        xc = x[c * BPC:(c + 1) * BPC].reshape(BPC, KC1, 128, N)
        xc = np.ascontiguousarray(xc.transpose(1, 2, 0, 3)).reshape(
            KC1, 128, BPC * N)
        in_maps.append(dict(shared, x=xc))
    return in_maps


def _run(in_maps, trace=False):
    from concourse.bass_utils import run_bass_kernel_spmd
    if "nc" not in _CACHE:
        _CACHE["nc"] = _build()
    nc = _CACHE["nc"]
    return run_bass_kernel_spmd(nc, in_maps, core_ids=list(range(NCORES)),
                                trace=trace)


def kernel(**inputs):
    in_maps = _prep_inputs(**inputs)
    res = _run(in_maps)
    out = np.empty((B, CIN, H, W), np.float32)
    for c in range(NCORES):
        yc = res.results[c]["y"].reshape(KC1, 128, BPC, N)
        out[c * BPC:(c + 1) * BPC] = yc.transpose(2, 0, 1, 3).reshape(
            BPC, CIN, H, W)
    return out
